# revision 49
# baseline (speedup 1.0000x reference)
"""Trainium2 Bass kernel v3 for nn_CombinedLoss (Poisson + 3-way pairwise CLIP).

Strategy (8 NeuronCores, SPMD, early AllGather + fp8 DoubleRow matmuls):
  - Row-shard the batch: core c owns rows [c*S, (c+1)*S) of every tensor.
  - Phase 0 per feature (order f2, f1, f3): load own rows, squared norms on
    ACT (Square accum), scale16 = 16/||z|| via Ln+Exp, DVE pre-scale to bf16,
    then PE identity-matmul transposes ([128,128] blocks into a 2-bank PSUM
    tile) and an ACT/DVE copy-cast to the fp8 zT layout [P, K, S]. No DMA
    transposes: the PE does them in ~150ns/block and the collectives are no
    longer serialized against transpose DMAs.
  - f2/f3's fp8 zT tiles are AllGathered (DRAM->DRAM, Shared output space)
    immediately after the feature is processed, so AG1 completes ~20us in and
    the matmul stream starts right after, overlapping AG2 and all remaining
    elementwise work.
  - Similarity blocks S_q = (16 Za)(16 Zb)^T = 256*sim via fp8e4 DoubleRow
    matmuls (2 k-tiles per pass), fp32 PSUM; exp(sim/T) = exp(S_q/128) as one
    1024-wide ACT activation per (pair, m, ntp) with accum_out row sums;
    column sums via DoubleRow ones-matmuls accumulated in PSUM.
  - Diagonal similarities are extracted from the sim PSUM tiles directly
    (DVE mult-by-identity + accum over each candidate 128-col block); the
    host picks the block matching the core id. No raw-feature dot products.
  - Poisson partials on own rows of inputs/targets overlap the matmul phase.
  - Host does only the O(B) final combine: log of row/col sums, means.
"""

import sys

import numpy as np

sys.path.insert(0, "/opt/trn_rl_repo")

P = 128
TEMPERATURE = 0.5
EPS_POISSON = 1e-8
LN16 = 2.772588722239781  # ln(16)
Q = 16.0  # fp8 pre-scale; S_q = Q^2 * sim


class Cfg:
    def __init__(self, B=4096, D=1024, n_cores=8):
        self.B = B
        self.D = D
        self.n_cores = n_cores
        self.S = B // n_cores       # own rows per core == columns per nt tile
        self.NTC = self.S
        self.MT = self.S // P       # own row tiles
        self.K = D // P             # k tiles (128-contraction)
        self.KD = self.K // 2       # DoubleRow k-pair count
        self.NT = B // self.NTC     # nt tiles == n_cores
        self.NTP = self.NT // 2     # nt tile pairs (one 2*NTC-wide exp each)
        assert B % n_cores == 0 and self.S % P == 0
        assert D % (2 * P) == 0 and self.NT % 2 == 0


def _patch_act_tables():
    """Make Bacc's act-table pass pick `natural_log_exp_and_others` for both
    Exp and Ln so alternating Ln/Exp/Square calls don't reload tables."""
    import functools

    import concourse.hw_specs as hw_specs

    if getattr(hw_specs, "_act_tables_patched", False):
        return
    orig = hw_specs.get_activation_tables

    @functools.cache
    def patched(module_arch):
        tabs = dict(orig(module_arch))
        names = list(tabs.keys())
        if "natural_log_exp_and_others" in tabs:
            combined = tabs["natural_log_exp_and_others"]
            for name in names:
                if name == "natural_log_exp_and_others":
                    break
                if tabs[name] & combined:
                    tabs[name] = tabs[name] - combined
        return tabs

    hw_specs.get_activation_tables = patched
    import concourse.bacc as bacc_mod

    if hasattr(bacc_mod, "get_activation_tables"):
        bacc_mod.get_activation_tables = patched
    hw_specs._act_tables_patched = True


def build_bass(cfg: Cfg, ag_space=None):
    if ag_space is None:
        # Shared-output HBM AllGather (direct peer writes) needs >4 cores.
        ag_space = "Shared" if cfg.n_cores > 4 else "Local"
    import concourse.bacc as bacc
    import concourse.bass as bass
    import concourse.mybir as mybir
    import concourse.tile as tile
    from concourse.masks import make_identity

    _patch_act_tables()

    f32 = mybir.dt.float32
    bf16 = mybir.dt.bfloat16
    fp8 = mybir.dt.float8e4
    AF = mybir.ActivationFunctionType
    ALU = mybir.AluOpType
    DR = mybir.MatmulPerfMode.DoubleRow
    ts = bass.ts

    B, D, K, KD, MT, NT, NTP, NTC, S, NC = (
        cfg.B, cfg.D, cfg.K, cfg.KD, cfg.MT, cfg.NT, cfg.NTP, cfg.NTC, cfg.S,
        cfg.n_cores,
    )
    RG = [list(range(NC))]
    PSC = max(D, 2 * NTC)  # psum tile cols: holds a K*P transpose or 2*NTC sim

    nc = bacc.Bacc(
        "TRN2",
        target_bir_lowering=False,
        debug=False,
        enable_asserts=False,
        num_devices=NC,
    )

    # ---- IO ----
    f1o = nc.dram_tensor("f1_own", [S, D], f32, kind="ExternalInput").ap()
    f2o = nc.dram_tensor("f2_own", [S, D], f32, kind="ExternalInput").ap()
    f3o = nc.dram_tensor("f3_own", [S, D], f32, kind="ExternalInput").ap()
    inp = nc.dram_tensor("inp_own", [S, D], f32, kind="ExternalInput").ap()
    tgt = nc.dram_tensor("tgt_own", [S, D], f32, kind="ExternalInput").ap()
    f_own = [f1o, f2o, f3o]

    rowparts_d = nc.dram_tensor("rowparts", [P, 3 * MT * NTP], f32, kind="ExternalOutput").ap()
    colparts_d = nc.dram_tensor("colparts", [1, 3 * B], f32, kind="ExternalOutput").ap()
    dots_d = nc.dram_tensor("dots_all", [P, 3 * MT * NT], f32, kind="ExternalOutput").ap()
    poi_d = nc.dram_tensor("poi", [P, 2 * MT], f32, kind="ExternalOutput").ap()

    with tile.TileContext(nc) as tc:
        with (
            tc.tile_pool(name="const", bufs=1) as const_pool,
            tc.tile_pool(name="persist", bufs=1) as persist,
            tc.tile_pool(name="rowf", bufs=8) as rowp,
            tc.tile_pool(name="zr16", bufs=3) as zrp,
            tc.tile_pool(name="junk", bufs=3) as junkp,
            tc.tile_pool(name="rhs", bufs=3) as rhsp,
            tc.tile_pool(name="exps", bufs=5) as expp,
            tc.tile_pool(name="small", bufs=4) as smallp,
            tc.tile_pool(name="colpp", bufs=2) as colpp,
            tc.tile_pool(name="dscr", bufs=1, space="DRAM") as dramp,
            tc.tile_pool(name="ps_s", bufs=2, space="PSUM") as ps_s,
            tc.tile_pool(name="ps_c", bufs=2, space="PSUM") as ps_c,
        ):
            ones8 = const_pool.tile([P, 2, P], fp8)
            nc.vector.memset(ones8, 1.0)
            ident = const_pool.tile([P, P], bf16)
            make_identity(nc, ident[:, :])
            eps_bias = const_pool.tile([P, 1], f32)
            nc.vector.memset(eps_bias, EPS_POISSON)
            ln16_bias = const_pool.tile([P, 1], f32)
            nc.vector.memset(ln16_bias, LN16)

            # persistent state
            zTq = [
                persist.tile([P, K, S], fp8, name=f"zTq{f}", tag=f"zTq{f}")
                for f in range(3)
            ]
            rowparts = persist.tile([P, 3 * MT * NTP], f32)
            dots = persist.tile([P, 3 * MT * NT], f32)
            poi = persist.tile([P, 2 * MT], f32)
            nsq = persist.tile([P, 3 * MT], f32)
            scale16 = persist.tile([P, 3 * MT], f32)

            agin = {}
            agout = {}
            for f in (1, 2):
                agin[f] = dramp.tile([P, K, S], fp8, tag=f"agin{f}", name=f"agin{f}")
                agout[f] = nc.dram_tensor(
                    f"agout{f}", [NC, P, K, S], fp8, addr_space=ag_space
                ).ap()

            def own_feature(f, do_ag):
                # per-tile pipeline: load -> Square (norms) -> per-tile
                # 16/||z|| scale -> DVE pre-scale to bf16 -> PE transpose
                # blocks into a PSUM tile -> copy-cast to fp8 zT layout ->
                # per-tile AllGather input chunk. The per-tile emission keeps
                # each step's semaphore wait tied to its own tile's load so
                # the AG input is ready as early as possible on every core.
                for t in range(MT):
                    rf = rowp.tile([P, D], f32, tag="rowf32")
                    nc.sync.dma_start(rf, f_own[f][ts(t, P), :])
                    jt = junkp.tile([P, D], bf16, tag="junk16")
                    slot = nsq[:, f * MT + t : f * MT + t + 1]
                    nc.scalar.activation(jt, rf, AF.Square, accum_out=slot)
                    l = smallp.tile([P, MT], f32, tag="lnsq")
                    nc.scalar.activation(l[:, 0:1], slot, AF.Ln)
                    # scale16 = exp(-0.5*ln(nsq) + ln 16) = 16/||z||
                    sslot = scale16[:, f * MT + t : f * MT + t + 1]
                    nc.scalar.activation(
                        sslot, l[:, 0:1], AF.Exp, scale=-0.5, bias=ln16_bias[:, :],
                    )
                    zr = zrp.tile([P, D], bf16, tag="zr16")
                    nc.vector.tensor_scalar_mul(zr, rf, sslot)
                    tps = ps_s.tile([P, PSC], f32, tag="ps")
                    for k in range(K):
                        nc.tensor.matmul(
                            tps[:, ts(k, P)], zr[:, ts(k, P)], ident,
                            start=True, stop=True,
                        )
                    # copy-cast [P, (k, j)] psum -> zTq[f][:, k, t*P + j] fp8
                    dst = zTq[f][:, :, ts(t, P)]
                    src = tps[:, 0:D]
                    if t % 2 == 0:
                        nc.scalar.activation(dst, src, AF.Copy)
                    else:
                        nc.vector.tensor_scalar_mul(dst, src, 1.0)
                    if do_ag:
                        nc.gpsimd.dma_start(agin[f][:, :, ts(t, P)], dst)
                if do_ag:
                    nc.gpsimd.collective_compute(
                        "AllGather",
                        ALU.bypass,
                        replica_groups=RG,
                        ins=[agin[f].opt()],
                        outs=[agout[f].opt()],
                    )

            own_feature(1, True)   # f2 first: its AG gates pairs 0 and 2
            own_feature(0, False)  # f1: lhsT only, needed by the first pairs
            own_feature(2, True)   # f3: gates pair 1 only (hidden under b=1)

            # ---- poisson partials (overlap AG2 + matmul stream) ----
            for t in range(MT):
                it = rowp.tile([P, D], f32, tag="rowf32")
                tt = rowp.tile([P, D], f32, tag="rowf32")
                nc.sync.dma_start(it, inp[ts(t, P), :])
                nc.sync.dma_start(tt, tgt[ts(t, P), :])
                lg = rowp.tile([P, D], f32, tag="rowf32")
                nc.scalar.activation(lg, it, AF.Ln, bias=eps_bias[:, :])
                jt = junkp.tile([P, D], bf16, tag="junk16")
                nc.vector.scalar_tensor_tensor(
                    out=jt, in0=tt, scalar=1.0, in1=lg,
                    op0=ALU.mult, op1=ALU.mult,
                    accum_out=poi[:, MT + t : MT + t + 1],
                )
                jt2 = junkp.tile([P, D], bf16, tag="junk16")
                nc.vector.tensor_scalar(
                    out=jt2, in0=it, scalar1=1.0, scalar2=0.0, op0=ALU.mult,
                    op1=ALU.add, accum_out=poi[:, t : t + 1],
                )

            # ---- phase 1: fp8 DoubleRow matmul stream over AG'd tiles ----
            # pair 2 = (f2, f3) is computed transposed (f3 rows x f2 cols):
            # loss_i + loss_j is symmetric in the transpose, so the host
            # combine is unchanged, and 2/3 of the matmuls depend only on
            # AG1 (f2), hiding AG2 entirely under the b=1 stream.
            partners_of = {1: [(0, 0), (2, 2)], 2: [(1, 0)]}
            pending = []

            def emit_colsum(grp):
                # column sums: fp8 DoubleRow ones-matmul reduces two m-tiles
                # (dim1 of the es pair-tile) per pass, PSUM-accumulated over
                # the m-pairs.
                pair, ntp, es_pairs = grp
                npair = len(es_pairs)
                cps = ps_c.tile([P, 2 * NTC], f32, tag="cps")
                for half in range(2):
                    for i, (esp, full) in enumerate(es_pairs):
                        if full:
                            nc.tensor.matmul(
                                cps[:, ts(half, NTC)], ones8,
                                esp[:, :, ts(half, NTC)],
                                start=(i == 0), stop=(i == npair - 1),
                                perf_mode=DR,
                            )
                        else:
                            nc.tensor.matmul(
                                cps[0:1, ts(half, NTC)], ones8[:, 0, 0:1],
                                esp[:, 0, ts(half, NTC)],
                                start=(i == 0), stop=(i == npair - 1),
                            )
                colp = colpp.tile([1, 2 * NTC], f32, tag="colp")
                nc.vector.tensor_scalar_mul(colp, cps[0:1, :], 1.0)
                nc.gpsimd.dma_start(
                    colparts_d[:, pair * B + ntp * 2 * NTC : pair * B + (ntp + 1) * 2 * NTC],
                    colp,
                )

            for b in (1, 2):
                for ntp in range(NTP):
                    zTr = rhsp.tile([P, 2, K, NTC], fp8, tag="zTr")
                    for i in range(2):
                        nc.sync.dma_start(zTr[:, i], agout[b][2 * ntp + i])
                    for (pair, a) in partners_of[b]:
                        es_pairs = []
                        for mp in range((MT + 1) // 2):
                            esp = expp.tile([P, 2, 2 * NTC], fp8, tag="es")
                            nsub = min(2, MT - 2 * mp)
                            for sub in range(nsub):
                                m = 2 * mp + sub
                                ps = ps_s.tile([P, PSC], f32, tag="ps")
                                # half-outer order: the first 4 matmuls only
                                # need the first 512KB zTr half, so the stream
                                # starts while the second half is in flight.
                                for half in range(2):
                                    for dk in range(KD):
                                        nc.tensor.matmul(
                                            ps[:, ts(half, NTC)],
                                            zTq[a][:, 2 * dk : 2 * dk + 2, ts(m, P)],
                                            zTr[:, half, 2 * dk : 2 * dk + 2, :],
                                            start=(dk == 0), stop=(dk == KD - 1),
                                            perf_mode=DR,
                                        )
                                slot = (pair * MT + m) * NTP + ntp
                                nc.scalar.activation(
                                    esp[:, sub, :], ps[:, 0 : 2 * NTC], AF.Exp,
                                    scale=2.0 / (Q * Q),
                                    accum_out=rowparts[:, slot : slot + 1],
                                )
                                # diag candidates: own-row dot lives in the
                                # (2*ntp+half == core) 128-col block; host
                                # selects. Read the fp8 es (SBUF) instead of
                                # the PSUM tile so the ps ring is released by
                                # the exp alone (fewer cross-engine syncs and
                                # no DVE PSUM reads). Slots hold exp(S_q/128);
                                # the host recovers sim via log.
                                for half in range(2):
                                    dj = junkp.tile([P, D], bf16, tag="junk16")
                                    dslot = (pair * MT + m) * NT + 2 * ntp + half
                                    nc.vector.scalar_tensor_tensor(
                                        out=dj[:, 0:P],
                                        in0=esp[:, sub, half * NTC + m * P : half * NTC + (m + 1) * P],
                                        scalar=1.0, in1=ident,
                                        op0=ALU.mult, op1=ALU.mult,
                                        accum_out=dots[:, dslot : dslot + 1],
                                    )
                            es_pairs.append((esp, nsub == 2))
                        pending.append((pair, ntp, es_pairs))
                        if len(pending) >= 2:
                            emit_colsum(pending.pop(0))
            for grp in pending:
                emit_colsum(grp)

            # ---- outputs ----
            nc.gpsimd.dma_start(rowparts_d, rowparts)
            nc.gpsimd.dma_start(dots_d, dots)
            nc.gpsimd.dma_start(poi_d, poi)

    nc.compile()
    return nc


def make_in_maps(cfg: Cfg, inputs, targets, feature1, feature2, feature3):
    f32 = np.float32
    ac = np.ascontiguousarray
    maps = []
    for c in range(cfg.n_cores):
        sl = slice(c * cfg.S, (c + 1) * cfg.S)
        maps.append({
            "f1_own": ac(feature1[sl], dtype=f32),
            "f2_own": ac(feature2[sl], dtype=f32),
            "f3_own": ac(feature3[sl], dtype=f32),
            "inp_own": ac(inputs[sl], dtype=f32),
            "tgt_own": ac(targets[sl], dtype=f32),
        })
    return maps


def combine_results(cfg: Cfg, per_core):
    B, MT, NT, NTP, S = cfg.B, cfg.MT, cfg.NT, cfg.NTP, cfg.S
    dots = np.zeros((3, B), np.float64)
    rowsum = np.zeros((3, B), np.float64)
    colsum = np.zeros((3, B), np.float64)
    poi_in = 0.0
    poi_tl = 0.0
    for c, r in enumerate(per_core):
        rp = np.asarray(r["rowparts"], np.float64)      # [128, 3*MT*NTP]
        cp = np.asarray(r["colparts"], np.float64)[0]   # [3*B]
        dt_ = np.asarray(r["dots_all"], np.float64)     # [128, 3*MT*NT]
        po = np.asarray(r["poi"], np.float64)           # [128, 2*MT]
        for pi in range(3):
            for m in range(MT):
                rows = slice(c * S + m * P, c * S + (m + 1) * P)
                # diag candidate block matching this core: slot holds
                # exp(S_q/128) -> sim = 0.5*ln(slot)
                dots[pi, rows] = 0.5 * np.log(dt_[:, (pi * MT + m) * NT + c])
                rowsum[pi, rows] = rp[:, (pi * MT + m) * NTP : (pi * MT + m + 1) * NTP].sum(axis=1)
            colsum[pi] += cp[pi * B : (pi + 1) * B]
        poi_in += po[:, :MT].sum()
        poi_tl += po[:, MT:].sum()

    pairs = ((0, 1), (0, 2), (1, 2))
    closs = 0.0
    for pi, (ia, ib) in enumerate(pairs):
        simdiag = dots[pi]
        loss_i = np.mean(np.log(rowsum[pi]) - simdiag / TEMPERATURE)
        loss_j = np.mean(np.log(colsum[pi]) - simdiag / TEMPERATURE)
        closs += 0.5 * (loss_i + loss_j)
    closs /= 3.0
    p_loss = (poi_in - poi_tl) / (cfg.B * cfg.D)
    total = p_loss + closs
    return (
        np.float32(total),
        np.float32(p_loss),
        np.float32(closs),
    )


_CACHE = {}


def _get_compiled(cfg: Cfg):
    key = (cfg.B, cfg.D, cfg.n_cores)
    if key not in _CACHE:
        _CACHE[key] = build_bass(cfg)
    return _CACHE[key]


def kernel(inputs, targets, feature1, feature2, feature3):
    from concourse.bass_utils import run_bass_kernel_spmd

    cfg = Cfg(B=inputs.shape[0], D=inputs.shape[1], n_cores=8)
    nc = _get_compiled(cfg)
    in_maps = make_in_maps(cfg, inputs, targets, feature1, feature2, feature3)
    res = run_bass_kernel_spmd(nc, in_maps, core_ids=list(range(cfg.n_cores)))
    return combine_results(cfg, res.results)


if __name__ == "__main__":
    rng = np.random.default_rng(0)
    B, D = 4096, 1024
    ins = {
        "inputs": rng.random((B, D), np.float32),
        "targets": rng.random((B, D), np.float32),
        "feature1": rng.standard_normal((B, D), np.float32),
        "feature2": rng.standard_normal((B, D), np.float32),
        "feature3": rng.standard_normal((B, D), np.float32),
    }
    out = kernel(**ins)
    print(out)
